# revision 13
# baseline (speedup 1.0000x reference)
"""Trainium2 Bass kernel for nn_ContrastiveLoss (SimCLR-style NT-Xent loss).

Reference computation (B=4096, D=256, T=0.07):
    f1, f2 = L2-normalize rows of features1/features2
    feats  = concat([f1, f2])                    # [8192, 256]
    sim    = feats @ feats.T / T                 # [8192, 8192]
    pos_i  = sim[i, (i+B) mod 2B]
    lse_i  = logsumexp over j != i of sim[i, j]
    loss   = mean(lse - pos)

Distribution: data-parallel over the 8192-row dimension across 8 NeuronCores.
Every core receives the FULL features (so the all-gather is free) plus its own
1024-row block ("ba") which becomes the stationary matmul operand.  Each core
computes sumexp over all 8192 columns for its row block, removes the diagonal
term exp(sim_ii) analytically (sim_ii recomputed per-row as the self-dot of
the *identical* fp16-normalized values that feed the matmul, so the
subtraction is consistent to fp32 rounding), takes the log, and writes per-row
lse.  The positive term is separable: mean(pos) over all rows = 2/N * sum over
the 4096 (f1_i . f2_i) pairs, which every core computes identically.  The host
sums the per-core partial outputs (the "all-reduce") and forms the scalar
loss.

Engine assignment (from timeline-sim profiling):
  - inputs are cast to fp16 on the host (layout/precision choice; all actual
    math runs on device) -> half the load bytes, DVE 2x-mode multiplies
  - squares and positive-pair products on GPSIMD (Pool), row-sums on DVE
  - 1/sqrt via seeded Newton iteration on DVE: ss ~ chi2(256) concentrates
    near 256, so a linear seed + 3 iterations reaches fp32 accuracy; this
    keeps the scalar engine on the single exp table set (no table switches)
  - d-major matmul operands produced by bouncing normalized rows through DRAM
    and reading back with one [1024, 128] -> [128, 1024] xbar DMA-transpose
    per group (large transposes amortize the per-instruction HWDGE overhead)
  - main loop: per (column-group, m): 8 matmuls (4 per LDWEIGHTS) into a
    [128, 2048] psum tile, then one fused exp + row-sum on the scalar engine
    reading psum directly (activation accum_out)
"""

import numpy as np

B = 4096
D = 256
N = 2 * B                  # 8192 total rows
NCORES = 8
RPC = N // NCORES          # 1024 rows per core
P = 128                    # SBUF partitions
TEMP = 0.07
WEIGHT = 1.0
TPG = 8                    # 8 x 128-row tiles = one 1024-row group
NGROUPS_F = 8              # f1: groups 0..3, f2: groups 4..7
GQ = 4                     # 4 column groups of 2048 for the main matmul
M_TILES = RPC // P         # 8 row tiles per core block

_cached_nc = None


def _build_module():
    import concourse.bacc as bacc
    import concourse.tile as tile
    import concourse.mybir as mybir

    f32 = mybir.dt.float32
    f16 = mybir.dt.float16
    AF = mybir.ActivationFunctionType
    AX = mybir.AxisListType
    ALU = mybir.AluOpType

    nc = bacc.Bacc(None, target_bir_lowering=False)

    f1 = nc.dram_tensor("f1", [B, D], f16, kind="ExternalInput")
    f2 = nc.dram_tensor("f2", [B, D], f16, kind="ExternalInput")
    ba = nc.dram_tensor("ba", [RPC, D], f16, kind="ExternalInput")
    out_lse = nc.dram_tensor("out_lse", [P, M_TILES], f32, kind="ExternalOutput")
    out_pos = nc.dram_tensor("out_pos", [P, 1], f32, kind="ExternalOutput")

    # DRAM views: group g covers rows [g*1024, (g+1)*1024) as [128p, 8t, 256d]
    f1_g = f1[:, :].rearrange("(g t p) d -> g p t d", p=P, t=TPG)
    f2_g = f2[:, :].rearrange("(g t p) d -> g p t d", p=P, t=TPG)
    ba_g = ba[:, :].rearrange("(t p) d -> p t d", p=P)

    with tile.TileContext(nc) as tc:
        with (
            tc.tile_pool(name="raw", bufs=1) as raw_pool,
            tc.tile_pool(name="prod", bufs=3) as prod_pool,
            tc.tile_pool(name="stats", bufs=6) as stats_pool,
            tc.tile_pool(name="persist", bufs=1) as persist,
            tc.tile_pool(name="expscr", bufs=4) as exp_pool,
            tc.tile_pool(name="small", bufs=1) as small,
            tc.tile_pool(name="dram", bufs=1, space="DRAM") as dram_pool,
            tc.tile_pool(name="psum", bufs=2, space="PSUM") as psum_pool,
        ):
            # featsT[k][gq]: [128(d half k), 2048(global rows)] fp16
            featsT = [
                [
                    persist.tile([P, 2048], f16, tag=f"fT{k}_{gq}", name=f"fT{k}_{gq}")
                    for gq in range(GQ)
                ]
                for k in range(2)
            ]
            lhsT = [
                persist.tile([P, RPC], f16, tag=f"lT{k}", name=f"lT{k}")
                for k in range(2)
            ]
            norm = [
                persist.tile([P, TPG, D], f16, tag=f"norm{g}", name=f"norm{g}")
                for g in range(NGROUPS_F + 1)
            ]
            ndram = [
                dram_pool.tile([RPC, D], f16, tag=f"ndram{g}", name=f"ndram{g}")
                for g in range(NGROUPS_F + 1)
            ]
            se_all = persist.tile([P, M_TILES * GQ], f32, tag="se_all")
            pos_all = persist.tile([P, 4 * TPG], f32, tag="pos_all")
            sd_tile = persist.tile([P, M_TILES], f32, tag="sd")

            # all 9 loads up front: the SP FIFO streams them back-to-back
            raws = {}
            order_srcs = [(8, ba_g)] + [(g, f1_g[g]) for g in range(4)] + [
                (4 + g, f2_g[g]) for g in range(4)
            ]
            for i, (g, src_ap) in enumerate(order_srcs):
                raw = raw_pool.tile(
                    [P, TPG, D], f16, tag=f"raw{g}", name=f"raw{g}", bufs=1
                )
                eng = nc.sync if i % 2 == 0 else nc.scalar
                eng.dma_start(out=raw[:], in_=src_ap)
                raws[g] = raw

            def process_group(g, fast=False):
                raw = raws[g]
                prod = prod_pool.tile([P, TPG, D], f16, tag="prod")
                # prologue-critical groups square on DVE (short latency);
                # steady-state groups use the otherwise-idle GPSIMD
                if fast:
                    nc.vector.tensor_mul(prod[:], raw[:], raw[:])
                else:
                    nc.gpsimd.tensor_mul(prod[:], raw[:], raw[:])
                ss = stats_pool.tile([P, TPG], f32, tag="ss")
                nc.vector.reduce_sum(ss[:], prod[:], axis=AX.X)
                rn = stats_pool.tile([P, TPG], f32, tag="rn")
                if fast:
                    # rn = 1/sqrt(ss) via ACT sqrt + DVE reciprocal: a 2-op
                    # chain; the sqrt-set table load sits in ACT's idle
                    # prologue window, well before the first exp
                    sq = stats_pool.tile([P, TPG], f32, tag="sq")
                    nc.scalar.activation(out=sq[:], in_=ss[:], func=AF.Sqrt)
                    nc.vector.reciprocal(rn[:], sq[:])
                else:
                    # Newton rsqrt on DVE (see module docstring)
                    nc.vector.tensor_scalar(
                        rn[:], ss[:], -1.0 / 8192.0, 3.0 / 32.0,
                        op0=ALU.mult, op1=ALU.add,
                    )
                    nt = stats_pool.tile([P, TPG], f32, tag="nt")
                    for _ in range(3):
                        nc.vector.tensor_mul(nt[:], rn[:], rn[:])
                        nc.vector.tensor_mul(nt[:], nt[:], ss[:])
                        nc.vector.tensor_scalar(
                            nt[:], nt[:], -0.5, 1.5, op0=ALU.mult, op1=ALU.add
                        )
                        nc.vector.tensor_mul(rn[:], rn[:], nt[:])
                rnh = stats_pool.tile([P, TPG], f16, tag="rnh")
                nc.vector.tensor_copy(rnh[:], rn[:])
                ng = norm[g]
                # fp16 x fp16 broadcast multiply -> DVE 2x mode
                nc.vector.tensor_tensor(
                    ng[:],
                    raw[:],
                    rnh[:, :, None].to_broadcast((P, TPG, D)),
                    ALU.mult,
                )
                # bounce to DRAM and read back transposed (both SP ring)
                nc.sync.dma_start(
                    out=ndram[g][:, :].rearrange("(t p) d -> p t d", p=P),
                    in_=ng[:],
                )
                for k in range(2):
                    if g < NGROUPS_F:
                        dst = featsT[k][g // 2][:, (g % 2) * RPC : (g % 2 + 1) * RPC]
                    else:
                        dst = lhsT[k][:]
                    nc.sync.dma_start_transpose(
                        out=dst, in_=ndram[g][:, k * P : (k + 1) * P]
                    )

            for i, (g, _) in enumerate(order_srcs):
                process_group(g, fast=(i < 3))

            # positives: rowwise dot(f1_hat_t, f2_hat_t) for all 4096 pairs
            for q in range(4):
                pp = prod_pool.tile([P, TPG, D], f16, tag="prod")
                nc.gpsimd.tensor_mul(pp[:], norm[q][:], norm[4 + q][:])
                nc.vector.reduce_sum(
                    pos_all[:, q * TPG : (q + 1) * TPG], pp[:], axis=AX.X
                )
            # self-dot of the core's own normalized fp16 rows (== matmul diag)
            sp = prod_pool.tile([P, TPG, D], f16, tag="prod")
            nc.gpsimd.tensor_mul(sp[:], norm[8][:], norm[8][:])
            nc.vector.reduce_sum(sd_tile[:], sp[:], axis=AX.X)

            # main matmul + fused exp/rowsum epilogue
            inv_t = float(1.0 / TEMP)
            for gq in range(GQ):
                for m in range(M_TILES):
                    ps = psum_pool.tile([P, 2048], f32, tag="ps")
                    for k in range(2):
                        for sl in range(4):
                            nc.tensor.matmul(
                                ps[:, sl * 512 : (sl + 1) * 512],
                                lhsT[k][:, m * P : (m + 1) * P],
                                featsT[k][gq][:, sl * 512 : (sl + 1) * 512],
                                start=(k == 0),
                                stop=(k == 1),
                            )
                    ex = exp_pool.tile([P, 2048], f16, tag="ex")
                    idx = m * GQ + gq
                    nc.scalar.activation(
                        out=ex[:],
                        in_=ps[:],
                        func=AF.Exp,
                        scale=inv_t,
                        accum_out=se_all[:, idx : idx + 1],
                    )

            # lse = log(sum_all - exp(selfdot/T))
            tot = small.tile([P, M_TILES], f32, tag="tot")
            nc.vector.reduce_sum(
                tot[:],
                se_all[:].rearrange("p (m q) -> p m q", q=GQ),
                axis=AX.X,
            )
            expd = small.tile([P, M_TILES], f32, tag="expd")
            nc.scalar.activation(out=expd[:], in_=sd_tile[:], func=AF.Exp, scale=inv_t)
            offs = small.tile([P, M_TILES], f32, tag="offs")
            nc.vector.tensor_sub(offs[:], tot[:], expd[:])
            lse = small.tile([P, M_TILES], f32, tag="lse")
            nc.scalar.activation(out=lse[:], in_=offs[:], func=AF.Ln)
            possum = small.tile([P, 1], f32, tag="possum")
            nc.vector.reduce_sum(possum[:], pos_all[:], axis=AX.X)

            nc.sync.dma_start(out=out_lse[:, :], in_=lse[:])
            nc.sync.dma_start(out=out_pos[:, :], in_=possum[:])

    nc.finalize()
    return nc


def _get_nc():
    global _cached_nc
    if _cached_nc is None:
        _cached_nc = _build_module()
    return _cached_nc


def _in_maps(features1, features2):
    f1 = np.ascontiguousarray(features1.astype(np.float16))
    f2 = np.ascontiguousarray(features2.astype(np.float16))
    feats = np.concatenate([f1, f2], axis=0)
    return [
        {
            "f1": f1,
            "f2": f2,
            "ba": np.ascontiguousarray(feats[c * RPC : (c + 1) * RPC]),
        }
        for c in range(NCORES)
    ]


def kernel(features1: np.ndarray, features2: np.ndarray) -> np.ndarray:
    from concourse.bass_utils import run_bass_kernel_spmd

    nc = _get_nc()
    res = run_bass_kernel_spmd(
        nc, _in_maps(features1, features2), core_ids=list(range(NCORES))
    )

    lse_total = 0.0
    for c in range(NCORES):
        lse_total += res.results[c]["out_lse"].astype(np.float64).sum()
    pos_raw = res.results[0]["out_pos"].astype(np.float64).sum()
    loss = (lse_total - 2.0 * pos_raw / TEMP) / N
    return np.array(WEIGHT * loss, dtype=np.float32)


# revision 16
# speedup vs baseline: 1.0653x; 1.0653x over previous
"""Trainium2 Bass kernel for nn_ContrastiveLoss (SimCLR-style NT-Xent loss).

Reference computation (B=4096, D=256, T=0.07):
    f1, f2 = L2-normalize rows of features1/features2
    feats  = concat([f1, f2])                    # [8192, 256]
    sim    = feats @ feats.T / T                 # [8192, 8192]
    pos_i  = sim[i, (i+B) mod 2B]
    lse_i  = logsumexp over j != i of sim[i, j]
    loss   = mean(lse - pos)

Distribution: data-parallel over the 8192-row dimension across 8 NeuronCores.
Every core receives the FULL features (so the all-gather is free) plus its own
1024-row block ("ba") which becomes the stationary matmul operand.  Each core
computes sumexp over all 8192 columns for its row block, removes the diagonal
term exp(sim_ii) analytically (sim_ii recomputed per-row as the self-dot of
the *identical* fp16-normalized values that feed the matmul, so the
subtraction is consistent to fp32 rounding), takes the log, and writes per-row
lse.  The positive term is separable: mean(pos) over all rows = 2/N * sum over
the 4096 (f1_i . f2_i) pairs, which every core computes identically.  The host
sums the per-core partial outputs (the "all-reduce") and forms the scalar
loss.

Engine assignment (from timeline-sim profiling):
  - inputs are cast to fp16 on the host (layout/precision choice; all actual
    math runs on device) -> half the load bytes, DVE 2x-mode multiplies
  - squares and positive-pair products on GPSIMD (Pool), row-sums on DVE
  - 1/sqrt via seeded Newton iteration on DVE: ss ~ chi2(256) concentrates
    near 256, so a linear seed + 3 iterations reaches fp32 accuracy; this
    keeps the scalar engine on the single exp table set (no table switches)
  - d-major matmul operands produced by bouncing normalized rows through DRAM
    and reading back with one [1024, 128] -> [128, 1024] xbar DMA-transpose
    per group (large transposes amortize the per-instruction HWDGE overhead)
  - main loop: per (column-group, m): 8 matmuls (4 per LDWEIGHTS) into a
    [128, 2048] psum tile, then one fused exp + row-sum on the scalar engine
    reading psum directly (activation accum_out)
"""

import numpy as np

B = 4096
D = 256
N = 2 * B                  # 8192 total rows
NCORES = 8
RPC = N // NCORES          # 1024 rows per core
P = 128                    # SBUF partitions
TEMP = 0.07
WEIGHT = 1.0
TPG = 8                    # 8 x 128-row tiles = one 1024-row group
NGROUPS_F = 8              # f1: groups 0..3, f2: groups 4..7
GQ = 4                     # 4 column groups of 2048 for the main matmul
M_TILES = RPC // P         # 8 row tiles per core block

_cached_nc = None


def _build_module():
    import concourse.bacc as bacc
    import concourse.tile as tile
    import concourse.mybir as mybir

    f32 = mybir.dt.float32
    f16 = mybir.dt.float16
    AF = mybir.ActivationFunctionType
    AX = mybir.AxisListType
    ALU = mybir.AluOpType

    nc = bacc.Bacc(None, target_bir_lowering=False)

    f1 = nc.dram_tensor("f1", [B, D], f16, kind="ExternalInput")
    f2 = nc.dram_tensor("f2", [B, D], f16, kind="ExternalInput")
    ba = nc.dram_tensor("ba", [RPC, D], f16, kind="ExternalInput")
    out_lse = nc.dram_tensor("out_lse", [P, M_TILES], f32, kind="ExternalOutput")
    out_pos = nc.dram_tensor("out_pos", [P, 1], f32, kind="ExternalOutput")

    # DRAM views: group g covers rows [g*1024, (g+1)*1024) as [128p, 8t, 256d]
    f1_g = f1[:, :].rearrange("(g t p) d -> g p t d", p=P, t=TPG)
    f2_g = f2[:, :].rearrange("(g t p) d -> g p t d", p=P, t=TPG)
    ba_g = ba[:, :].rearrange("(t p) d -> p t d", p=P)

    with tile.TileContext(nc) as tc:
        with (
            tc.tile_pool(name="raw", bufs=1) as raw_pool,
            tc.tile_pool(name="prod", bufs=3) as prod_pool,
            tc.tile_pool(name="stats", bufs=6) as stats_pool,
            tc.tile_pool(name="persist", bufs=1) as persist,
            tc.tile_pool(name="expscr", bufs=4) as exp_pool,
            tc.tile_pool(name="small", bufs=1) as small,
            tc.tile_pool(name="dram", bufs=1, space="DRAM") as dram_pool,
            tc.tile_pool(name="psum", bufs=2, space="PSUM") as psum_pool,
        ):
            # featsT[k][gq]: [128(d half k), 2048(global rows)] fp16
            featsT = [
                [
                    persist.tile([P, 2048], f16, tag=f"fT{k}_{gq}", name=f"fT{k}_{gq}")
                    for gq in range(GQ)
                ]
                for k in range(2)
            ]
            lhsT = [
                persist.tile([P, RPC], f16, tag=f"lT{k}", name=f"lT{k}")
                for k in range(2)
            ]
            norm = [
                persist.tile([P, TPG, D], f16, tag=f"norm{g}", name=f"norm{g}")
                for g in range(NGROUPS_F + 1)
            ]
            ndram = [
                dram_pool.tile([RPC, D], f16, tag=f"ndram{g}", name=f"ndram{g}")
                for g in range(NGROUPS_F + 1)
            ]
            se_all = persist.tile([P, M_TILES * GQ], f32, tag="se_all")
            pos_all = persist.tile([P, 4 * TPG], f32, tag="pos_all")
            sd_tile = persist.tile([P, M_TILES], f32, tag="sd")

            # all 9 loads up front: the SP FIFO streams them back-to-back
            raws = {}
            order_srcs = [(8, ba_g)] + [(g, f1_g[g]) for g in range(4)] + [
                (4 + g, f2_g[g]) for g in range(4)
            ]
            for i, (g, src_ap) in enumerate(order_srcs):
                raw = raw_pool.tile(
                    [P, TPG, D], f16, tag=f"raw{g}", name=f"raw{g}", bufs=1
                )
                eng = nc.sync if i % 2 == 0 else nc.scalar
                eng.dma_start(out=raw[:], in_=src_ap)
                raws[g] = raw

            def process_group(g, fast=False):
                raw = raws[g]
                prod = prod_pool.tile([P, TPG, D], f16, tag="prod")
                # prologue-critical groups square on DVE (short latency);
                # steady-state groups use the otherwise-idle GPSIMD
                if fast:
                    nc.vector.tensor_mul(prod[:], raw[:], raw[:])
                else:
                    nc.gpsimd.tensor_mul(prod[:], raw[:], raw[:])
                ss = stats_pool.tile([P, TPG], f32, tag="ss")
                nc.vector.reduce_sum(ss[:], prod[:], axis=AX.X)
                rn = stats_pool.tile([P, TPG], f32, tag="rn")
                if fast:
                    # rn = 1/sqrt(ss) via ACT sqrt + DVE reciprocal: a 2-op
                    # chain; the sqrt-set table load sits in ACT's idle
                    # prologue window, well before the first exp
                    sq = stats_pool.tile([P, TPG], f32, tag="sq")
                    nc.scalar.activation(out=sq[:], in_=ss[:], func=AF.Sqrt)
                    nc.vector.reciprocal(rn[:], sq[:])
                else:
                    # Newton rsqrt on DVE (see module docstring)
                    nc.vector.tensor_scalar(
                        rn[:], ss[:], -1.0 / 8192.0, 3.0 / 32.0,
                        op0=ALU.mult, op1=ALU.add,
                    )
                    nt = stats_pool.tile([P, TPG], f32, tag="nt")
                    for _ in range(3):
                        nc.vector.tensor_mul(nt[:], rn[:], rn[:])
                        nc.vector.tensor_mul(nt[:], nt[:], ss[:])
                        nc.vector.tensor_scalar(
                            nt[:], nt[:], -0.5, 1.5, op0=ALU.mult, op1=ALU.add
                        )
                        nc.vector.tensor_mul(rn[:], rn[:], nt[:])
                ng = norm[g]
                # per-tile scalar multiply: fp16 src + immediate-free scalar
                # AP runs in the DVE 4x perf mode (the stride-0 broadcast
                # tensor_tensor alternative is locked to 1x)
                for t in range(TPG):
                    nc.vector.tensor_scalar_mul(
                        ng[:, t, :], raw[:, t, :], rn[:, t : t + 1]
                    )
                # bounce to DRAM and read back transposed (both SP ring)
                nc.sync.dma_start(
                    out=ndram[g][:, :].rearrange("(t p) d -> p t d", p=P),
                    in_=ng[:],
                )
                for k in range(2):
                    if g < NGROUPS_F:
                        dst = featsT[k][g // 2][:, (g % 2) * RPC : (g % 2 + 1) * RPC]
                    else:
                        dst = lhsT[k][:]
                    if fast:
                        half = RPC // 2
                        for h in range(2):
                            nc.sync.dma_start_transpose(
                                out=dst[:, h * half : (h + 1) * half],
                                in_=ndram[g][
                                    h * half : (h + 1) * half,
                                    k * P : (k + 1) * P,
                                ],
                            )
                    else:
                        nc.sync.dma_start_transpose(
                            out=dst, in_=ndram[g][:, k * P : (k + 1) * P]
                        )

            for i, (g, _) in enumerate(order_srcs):
                process_group(g, fast=(i < 3))

            # positives: rowwise dot(f1_hat_t, f2_hat_t) for all 4096 pairs
            for q in range(4):
                pp = prod_pool.tile([P, TPG, D], f16, tag="prod")
                nc.gpsimd.tensor_mul(pp[:], norm[q][:], norm[4 + q][:])
                nc.vector.reduce_sum(
                    pos_all[:, q * TPG : (q + 1) * TPG], pp[:], axis=AX.X
                )
            # self-dot of the core's own normalized fp16 rows (== matmul diag)
            sp = prod_pool.tile([P, TPG, D], f16, tag="prod")
            nc.gpsimd.tensor_mul(sp[:], norm[8][:], norm[8][:])
            nc.vector.reduce_sum(sd_tile[:], sp[:], axis=AX.X)

            # main matmul + fused exp/rowsum epilogue
            inv_t = float(1.0 / TEMP)
            for gq in range(GQ):
                for m in range(M_TILES):
                    ps = psum_pool.tile([P, 2048], f32, tag="ps")
                    for k in range(2):
                        for sl in range(4):
                            nc.tensor.matmul(
                                ps[:, sl * 512 : (sl + 1) * 512],
                                lhsT[k][:, m * P : (m + 1) * P],
                                featsT[k][gq][:, sl * 512 : (sl + 1) * 512],
                                start=(k == 0),
                                stop=(k == 1),
                            )
                    ex = exp_pool.tile([P, 2048], f16, tag="ex")
                    idx = m * GQ + gq
                    nc.scalar.activation(
                        out=ex[:],
                        in_=ps[:],
                        func=AF.Exp,
                        scale=inv_t,
                        accum_out=se_all[:, idx : idx + 1],
                    )

            # lse = log(sum_all - exp(selfdot/T))
            tot = small.tile([P, M_TILES], f32, tag="tot")
            nc.vector.reduce_sum(
                tot[:],
                se_all[:].rearrange("p (m q) -> p m q", q=GQ),
                axis=AX.X,
            )
            expd = small.tile([P, M_TILES], f32, tag="expd")
            nc.scalar.activation(out=expd[:], in_=sd_tile[:], func=AF.Exp, scale=inv_t)
            offs = small.tile([P, M_TILES], f32, tag="offs")
            nc.vector.tensor_sub(offs[:], tot[:], expd[:])
            lse = small.tile([P, M_TILES], f32, tag="lse")
            nc.scalar.activation(out=lse[:], in_=offs[:], func=AF.Ln)
            possum = small.tile([P, 1], f32, tag="possum")
            nc.vector.reduce_sum(possum[:], pos_all[:], axis=AX.X)

            nc.sync.dma_start(out=out_lse[:, :], in_=lse[:])
            nc.sync.dma_start(out=out_pos[:, :], in_=possum[:])

    nc.finalize()
    return nc


def _get_nc():
    global _cached_nc
    if _cached_nc is None:
        _cached_nc = _build_module()
    return _cached_nc


def _in_maps(features1, features2):
    f1 = np.ascontiguousarray(features1.astype(np.float16))
    f2 = np.ascontiguousarray(features2.astype(np.float16))
    feats = np.concatenate([f1, f2], axis=0)
    return [
        {
            "f1": f1,
            "f2": f2,
            "ba": np.ascontiguousarray(feats[c * RPC : (c + 1) * RPC]),
        }
        for c in range(NCORES)
    ]


def kernel(features1: np.ndarray, features2: np.ndarray) -> np.ndarray:
    from concourse.bass_utils import run_bass_kernel_spmd

    nc = _get_nc()
    res = run_bass_kernel_spmd(
        nc, _in_maps(features1, features2), core_ids=list(range(NCORES))
    )

    lse_total = 0.0
    for c in range(NCORES):
        lse_total += res.results[c]["out_lse"].astype(np.float64).sum()
    pos_raw = res.results[0]["out_pos"].astype(np.float64).sum()
    loss = (lse_total - 2.0 * pos_raw / TEMP) / N
    return np.array(WEIGHT * loss, dtype=np.float32)
